# revision 1
# baseline (speedup 1.0000x reference)
"""Trainium2 Bass kernel for nn_Decoder_Block_Color (dense transformer block).

Strategy: pure data parallelism over batch (16 batches -> 8 cores x 2).
Each core runs the full decoder block on its 2 batch elements.

Dataflow is fully "transposed": activations live as [channel, token] so every
linear layer is matmul(out=psum, lhsT=W[c_in, c_out], rhs=actT[c_in, tok]) with
no activation transposes anywhere.  Per-token reductions (LN stats, softmax
denominator) are done on the PE with ones-column matmuls; per-token broadcasts
(LN scale, softmax normalization) are built as K=1/K=2 outer-product matmuls in
f32r.  Attention scores are computed directly in [tok_k, tok_q] layout
(lhsT = kT head slice), so softmax needs no max-subtraction (scores are small
by construction) and P@V consumes the exp matrix without any transpose; the
softmax sums ride along as a 65th lhsT column of ones in the P@V matmul.

The mask is preprocessed on the host into a transposed 0/1 fp8 tensor (padded
to 1408 rows); exp(scale*scores) is multiplied by it on the vector engine.
"""

import numpy as np

_B, _N, _L, _C, _H, _MLP = 16, 1024, 313, 768, 12, 3072
_DH = _C // _H            # 64
_NL = _N + _L             # 1337
_NLP = 1408               # 11 * 128 (padded key/value tokens)
_P = 128
_KC = _C // _P            # 6
_KM = _MLP // _P          # 24
_NCORES = 8
_BPC = _B // _NCORES      # 2 batches per core
_EPS = 1e-5
_SCALE = _DH ** -0.5
_NKT = _NLP // _P         # 11 key-token tiles
_MASK_DT = "fp8"          # "fp8" or "bf16"
_TRACE = False            # set True from test harness for NTFF profiling


def _split_multi_waits(nc):
    """Rewrite blocks so every instruction carries at most one sync wait.

    The walrus in this container rejects >1 sync wait per instruction
    ("Too many sync wait commands") while Tile's semaphore assignment
    attaches several; extra waits move to engine-matched nops inserted
    immediately before the instruction.
    """
    from concourse import mybir

    cur = nc.cur_bb
    assert cur is not None
    cur_list = cur.bb.instructions

    def make_nop(engine, wait):
        nop = nc.engines[engine].nop(nofuse=True).ins
        popped = cur_list.pop()
        assert popped is nop, (popped.name, nop.name)
        nop.sync_info = mybir.SyncInfo(on_wait=[wait], on_update=[])
        return nop

    for f in nc.m.functions:
        for blk in f.blocks:
            insts = blk.instructions
            if not any(
                i.sync_info and i.sync_info.on_wait and len(i.sync_info.on_wait) > 1
                for i in insts
            ):
                continue
            out = []
            for inst in insts:
                si = inst.sync_info
                waits = list(si.on_wait) if (si and si.on_wait) else []
                if len(waits) > 1:
                    for w in waits[:-1]:
                        out.append(make_nop(inst.engine, w))
                    inst.sync_info = mybir.SyncInfo(
                        on_wait=[waits[-1]], on_update=list(si.on_update or [])
                    )
                out.append(inst)
            blk.instructions[:] = out


def _build():
    import concourse.bass as bass
    import concourse.tile as tile
    from concourse import mybir
    from concourse.bass import ts
    from contextlib import ExitStack

    f32 = mybir.dt.float32
    f32r = mybir.dt.float32r
    bf16 = mybir.dt.bfloat16
    fp8 = mybir.dt.float8e4
    mask_dt = fp8 if _MASK_DT == "fp8" else bf16
    AF = mybir.ActivationFunctionType
    OP = mybir.AluOpType

    nc = bass.Bass()

    # ---- DRAM I/O ----
    xT_d = nc.dram_tensor("xT", (_BPC, _C, _N), bf16, kind="ExternalInput")
    ceT_d = nc.dram_tensor("ceT", (_BPC, _C, _L), bf16, kind="ExternalInput")
    maskT_d = nc.dram_tensor("maskT", (_BPC, _NLP, _N), mask_dt, kind="ExternalInput")
    Wq_d = nc.dram_tensor("Wq", (_C, _C), bf16, kind="ExternalInput")
    Wk_d = nc.dram_tensor("Wk", (_C, _C), bf16, kind="ExternalInput")
    Wv_d = nc.dram_tensor("Wv", (_C, _C), bf16, kind="ExternalInput")
    Wp_d = nc.dram_tensor("Wp", (_C, _C), bf16, kind="ExternalInput")
    W1_d = nc.dram_tensor("W1", (_C, _MLP), bf16, kind="ExternalInput")
    W2_d = nc.dram_tensor("W2", (_MLP, _C), bf16, kind="ExternalInput")
    bq_d = nc.dram_tensor("bq", (_C,), f32, kind="ExternalInput")
    bk_d = nc.dram_tensor("bk", (_C,), f32, kind="ExternalInput")
    bv_d = nc.dram_tensor("bv", (_C,), f32, kind="ExternalInput")
    bp_d = nc.dram_tensor("bp", (_C,), f32, kind="ExternalInput")
    b1_d = nc.dram_tensor("b1", (_MLP,), f32, kind="ExternalInput")
    b2_d = nc.dram_tensor("b2", (_C,), f32, kind="ExternalInput")
    g1_d = nc.dram_tensor("g1", (_C,), f32, kind="ExternalInput")
    be1_d = nc.dram_tensor("be1", (_C,), f32, kind="ExternalInput")
    g2_d = nc.dram_tensor("g2", (_C,), f32, kind="ExternalInput")
    be2_d = nc.dram_tensor("be2", (_C,), f32, kind="ExternalInput")
    outT_d = nc.dram_tensor("outT", (_BPC, _C, _N), f32, kind="ExternalOutput")

    def ptile(ap2d):  # [(t p), m] dram view -> [p, t, m]
        return ap2d.rearrange("(t p) m -> p t m", p=_P)

    def pcols(ap1d):  # [(t p)] dram view -> [p, t]
        return ap1d.rearrange("(t p) -> p t", p=_P)

    with tile.TileContext(nc) as tc, ExitStack() as ctx:
        sing = ctx.enter_context(tc.tile_pool(name="sing", bufs=1))
        dbl = ctx.enter_context(tc.tile_pool(name="dbl", bufs=2))
        tri = ctx.enter_context(tc.tile_pool(name="tri", bufs=3))

        # ---- constants / biases (persistent) ----
        bq_sb = sing.tile([_P, _KC], f32, tag="bq")
        nc.sync.dma_start(bq_sb[:], pcols(bq_d[:]))
        bk_sb = sing.tile([_P, _KC], f32, tag="bk")
        nc.sync.dma_start(bk_sb[:], pcols(bk_d[:]))
        bp_sb = sing.tile([_P, _KC], f32, tag="bp")
        nc.sync.dma_start(bp_sb[:], pcols(bp_d[:]))
        b2_sb = sing.tile([_P, _KC], f32, tag="b2")
        nc.sync.dma_start(b2_sb[:], pcols(b2_d[:]))
        b1_sb = sing.tile([_P, _KM], f32, tag="b1")
        nc.sync.dma_start(b1_sb[:], pcols(b1_d[:]))
        be1_sb = sing.tile([_P, _KC], f32, tag="be1")
        nc.sync.dma_start(be1_sb[:], pcols(be1_d[:]))
        be2_sb = sing.tile([_P, _KC], f32, tag="be2")
        nc.sync.dma_start(be2_sb[:], pcols(be2_d[:]))
        g1_rowf = sing.tile([1, _C], f32, tag="growf")
        nc.sync.dma_start(g1_rowf[:], g1_d[None, :])
        g1_row = sing.tile([1, _C], f32r, tag="g1row")
        with nc.allow_low_precision(reason="f32r broadcast operand"):
            nc.vector.tensor_copy(g1_row[:], g1_rowf[:])
        g2_rowf = sing.tile([1, _C], f32, tag="growf")
        nc.sync.dma_start(g2_rowf[:], g2_d[None, :])
        g2_row = sing.tile([1, _C], f32r, tag="g2row")
        with nc.allow_low_precision(reason="f32r broadcast operand"):
            nc.vector.tensor_copy(g2_row[:], g2_rowf[:])
        bv_bc = sing.tile([_P, _C], f32, tag="bvbc")
        nc.gpsimd.dma_start(bv_bc[:], bv_d[None, :].to_broadcast((_P, _C)))
        ones_b = sing.tile([_P, 1], bf16, tag="onesb")
        nc.vector.memset(ones_b[:], 1.0)
        ones_f32 = sing.tile([_P, 1], f32, tag="onesf32")
        nc.vector.memset(ones_f32[:], 1.0)
        ones_f = sing.tile([_P, 1], f32r, tag="onesf")
        onesrow_f32 = sing.tile([1, _P], f32, tag="onesrowf32")
        nc.vector.memset(onesrow_f32[:], 1.0)
        ones_row = sing.tile([1, _P], f32r, tag="onesrow")
        with nc.allow_low_precision(reason="f32r ones"):
            nc.vector.tensor_copy(ones_f[:], ones_f32[:])
            nc.vector.tensor_copy(ones_row[:], onesrow_f32[:])
        eps_sb = sing.tile([1, 1], f32, tag="eps")
        nc.vector.memset(eps_sb[:], _EPS)

        def layer_norm_T(psum, src_tile, g_row, be_sb, out_writer):
            """src_tile(t) -> [128, N] bf16 AP; writes normalized bf16 out."""
            s1 = psum.tile([1, _N], f32, tag="s1")
            s2 = psum.tile([1, _N], f32, tag="s2")
            for t in range(_KC):
                xt = src_tile(t)
                sq = dbl.tile([_P, _N], f32r, tag="t4k")
                with nc.allow_low_precision(reason="f32r stats operand"):
                    nc.vector.tensor_mul(sq[:], xt, xt)
                for c in range(2):
                    nc.tensor.matmul(
                        s1[:, ts(c, 512)], ones_b[:], xt[:, ts(c, 512)],
                        start=(t == 0), stop=(t == _KC - 1))
                    nc.tensor.matmul(
                        s2[:, ts(c, 512)], ones_f[:],
                        sq[:, ts(c, 512)],
                        start=(t == 0), stop=(t == _KC - 1))
            # rstd and mu*rstd feed matmuls whose operands must share the
            # lhsT base partition (0), so they live on partition 0.
            mmrows = sing.tile([1, 2, _N], f32r, tag="lnmmrows")
            rstd = mmrows[:, 0, :]
            mr = mmrows[:, 1, :]
            rows = sing.tile([1, 2, _N], f32, tag="lnrows")
            mu = rows[:, 0, :]
            nc.vector.tensor_scalar_mul(mu, s1[:], 1.0 / _C)
            mumu = rows[:, 1, :]
            nc.vector.tensor_mul(mumu, mu, mu)
            var = rows[:, 1, :]
            nc.vector.scalar_tensor_tensor(
                var, s2[:], 1.0 / _C, mumu, op0=OP.mult, op1=OP.subtract)
            std = rows[:, 1, :]
            nc.scalar.activation(std, var, AF.Sqrt, bias=eps_sb[:], scale=1.0)
            with nc.allow_low_precision(reason="f32r broadcast operand"):
                nc.vector.reciprocal(rstd, std)
                nc.vector.tensor_mul(mr, mu, rstd)
            for t in range(_KC):
                A = psum.tile([_P, _N], f32, tag="lnA")
                Bp = psum.tile([_P, _N], f32, tag="lnB")
                for c in range(2):
                    nc.tensor.matmul(
                        A[:, ts(c, 512)],
                        g_row[:, ts(t, _P)],
                        rstd[:, ts(c, 512)],
                        start=True, stop=True)
                    nc.tensor.matmul(
                        Bp[:, ts(c, 512)],
                        g_row[:, ts(t, _P)],
                        mr[:, ts(c, 512)],
                        start=True, stop=True)
                t1 = dbl.tile([_P, _N], f32, tag="t4k")
                nc.vector.tensor_mul(t1[:], src_tile(t), A[:])
                # out = (t1 + beta[c]) - g*mu*rstd
                out_writer(t, t1, Bp, be_sb)

        for b in range(_BPC):
            # =========== LN1 ===========
            xT = sing.tile([_P, _KC, _N], bf16, tag="xT")
            nc.sync.dma_start(xT[:], ptile(xT_d[b]))
            tokensT = dbl.tile([_P, _KC, _NLP], bf16, tag="tk17")
            nc.vector.memset(tokensT[:, :, _NL:], 0.0)
            nc.sync.dma_start(tokensT[:, :, _N:_NL], ptile(ceT_d[b]))

            with tc.tile_pool(name=f"psA{b}", bufs=1, space="PSUM") as psA:
                def w_ln1(t, t1, Bp, be_sb):
                    nc.vector.scalar_tensor_tensor(
                        tokensT[:, t, 0:_N], t1[:], be_sb[:, t:t + 1], Bp[:],
                        op0=OP.add, op1=OP.subtract)
                layer_norm_T(psA, lambda t: xT[:, t, :], g1_row, be1_sb, w_ln1)

            # =========== QKV ===========
            qT = dbl.tile([_P, _KC, _N], bf16, tag="q12")
            kT = dbl.tile([_P, _KC, _NLP], bf16, tag="tk17")
            vaug = sing.tile([_P, _NKT, _H, 65], bf16, tag="vaug")
            nc.vector.memset(vaug[:, :, :, 64:65], 1.0)

            with tc.tile_pool(name=f"psB{b}", bufs=4, space="PSUM") as psB:
                Wq_sb = dbl.tile([_P, _KC, _C], bf16, tag="w9")
                nc.sync.dma_start(Wq_sb[:], ptile(Wq_d[:]))
                for mt in range(_KC):
                    for c in range(2):
                        ps = psB.tile([_P, 512], f32, tag="qkv")
                        for kt in range(_KC):
                            nc.tensor.matmul(
                                ps[:], Wq_sb[:, kt, ts(mt, _P)],
                                tokensT[:, kt, ts(c, 512)],
                                start=(kt == 0), stop=(kt == _KC - 1))
                        nc.scalar.activation(
                            qT[:, mt, ts(c, 512)], ps[:], AF.Identity,
                            bias=bq_sb[:, mt:mt + 1], scale=1.0)
                Wk_sb = dbl.tile([_P, _KC, _C], bf16, tag="w9")
                nc.sync.dma_start(Wk_sb[:], ptile(Wk_d[:]))
                kchunks = [(0, 512), (512, 512), (1024, 384)]
                for mt in range(_KC):
                    for off, sz in kchunks:
                        ps = psB.tile([_P, 512], f32, tag="qkv")
                        for kt in range(_KC):
                            nc.tensor.matmul(
                                ps[:, :sz], Wk_sb[:, kt, ts(mt, _P)],
                                tokensT[:, kt, off:off + sz],
                                start=(kt == 0), stop=(kt == _KC - 1))
                        nc.scalar.activation(
                            kT[:, mt, off:off + sz], ps[:, :sz], AF.Identity,
                            bias=bk_sb[:, mt:mt + 1], scale=1.0)
                Wv_sb = dbl.tile([_P, _KC, _C], bf16, tag="w9")
                nc.sync.dma_start(Wv_sb[:], ptile(Wv_d[:]))
                vchunks = [(0, 512, 0, 8), (512, 256, 8, 4)]
                for tk in range(_NKT):
                    for off, sz, h0, nh in vchunks:
                        ps = psB.tile([_P, 512], f32, tag="qkv")
                        for kt in range(_KC):
                            nc.tensor.matmul(
                                ps[:, :sz], tokensT[:, kt, ts(tk, _P)],
                                Wv_sb[:, kt, off:off + sz],
                                start=(kt == 0), stop=(kt == _KC - 1))
                        nc.vector.tensor_tensor(
                            vaug[:, tk, h0:h0 + nh, 0:64],
                            ps[:, :sz].rearrange("p (h d) -> p h d", d=_DH),
                            bv_bc[:, off:off + sz].rearrange(
                                "p (h d) -> p h d", d=_DH),
                            op=OP.add)

            # =========== attention ===========
            maskT = sing.tile([_P, _NKT, _N], mask_dt, tag="maskT")
            nc.sync.dma_start(
                maskT[:], maskT_d[b].rearrange("(t p) n -> p t n", p=_P))
            attn_outT = dbl.tile([_P, _KC, _N], bf16, tag="q12")

            with tc.tile_pool(name=f"psC{b}", bufs=2, space="PSUM") as psC:
                for t in range(_KC):  # head pair (2t, 2t+1)
                    outUs = []
                    for e in range(2):
                        outUs.append(psC.tile([_P, _N], f32, tag="outU", name=f"outU{e}"))
                    for tk in range(_NKT):
                        sps = []
                        for e in range(2):
                            sp = psC.tile([_P, _N], f32, tag="scores")
                            rows = slice(64 * e, 64 * e + 64)
                            for c in range(2):
                                nc.tensor.matmul(
                                    sp[:, ts(c, 512)],
                                    kT[rows, t, ts(tk, _P)],
                                    qT[rows, t, ts(c, 512)],
                                    start=True, stop=True,
                                    tile_position=(64 * e, 0))
                            sps.append(sp)
                        for e in range(2):
                            pT = dbl.tile([_P, _N], bf16, tag="pT")
                            nc.scalar.activation(
                                pT[:], sps[e][:], AF.Exp, bias=0.0,
                                scale=_SCALE)
                            nc.vector.tensor_mul(
                                pT[:], pT[:], maskT[:, tk, :])
                            for c in range(2):
                                nc.tensor.matmul(
                                    outUs[e][0:65, ts(c, 512)],
                                    vaug[:, tk, 2 * t + e, :],
                                    pT[:, ts(c, 512)],
                                    start=(tk == 0), stop=(tk == _NKT - 1))
                    recips = sing.tile([1, 2, _N], f32r, tag="recips")
                    with nc.allow_low_precision(reason="f32r softmax recip"):
                        for e in range(2):
                            nc.vector.reciprocal(
                                recips[:, e, :], outUs[e][64:65, :])
                    for e in range(2):
                        D = psC.tile([_P, _N], f32, tag="scores",
                                     name=f"D{e}")
                        for c in range(2):
                            nc.tensor.matmul(
                                D[:, ts(c, 512)], ones_row[:],
                                recips[:, e, ts(c, 512)],
                                start=True, stop=True)
                        rows = slice(64 * e, 64 * e + 64)
                        nc.scalar.activation(
                            attn_outT[rows, t, :], outUs[e][0:64, :],
                            AF.Copy, bias=0.0, scale=1.0)
                        nc.vector.tensor_mul(
                            attn_outT[rows, t, :], attn_outT[rows, t, :],
                            D[0:64, :])

            # =========== proj + residual ===========
            x1T = sing.tile([_P, _KC, _N], bf16, tag="x1T")
            with tc.tile_pool(name=f"psD{b}", bufs=4, space="PSUM") as psD:
                Wp_sb = dbl.tile([_P, _KC, _C], bf16, tag="w9")
                nc.sync.dma_start(Wp_sb[:], ptile(Wp_d[:]))
                for mt in range(_KC):
                    for c in range(2):
                        ps = psD.tile([_P, 512], f32, tag="proj")
                        for kt in range(_KC):
                            nc.tensor.matmul(
                                ps[:], Wp_sb[:, kt, ts(mt, _P)],
                                attn_outT[:, kt, ts(c, 512)],
                                start=(kt == 0), stop=(kt == _KC - 1))
                        nc.vector.scalar_tensor_tensor(
                            x1T[:, mt, ts(c, 512)], ps[:],
                            bp_sb[:, mt:mt + 1], tokensT[:, mt, ts(c, 512)],
                            op0=OP.add, op1=OP.add)

            # =========== LN2 ===========
            ln2T = dbl.tile([_P, _KC, _N], bf16, tag="q12")
            with tc.tile_pool(name=f"psE{b}", bufs=1, space="PSUM") as psE:
                def w_ln2(t, t1, Bp, be_sb):
                    nc.vector.scalar_tensor_tensor(
                        ln2T[:, t, :], t1[:], be_sb[:, t:t + 1], Bp[:],
                        op0=OP.add, op1=OP.subtract)
                layer_norm_T(psE, lambda t: x1T[:, t, :], g2_row, be2_sb, w_ln2)

            # =========== MLP ===========
            hT = sing.tile([_P, _KM, 512], bf16, tag="hT")
            with tc.tile_pool(name=f"psF{b}", bufs=2, space="PSUM") as psF, \
                 tc.tile_pool(name=f"psG{b}", bufs=6, space="PSUM") as psG:
                for c in range(2):
                    for g in range(4):  # W1 column group: mt 6g..6g+5
                        W1_sb = dbl.tile([_P, _KC, _C], bf16, tag="w9")
                        nc.sync.dma_start(
                            W1_sb[:], ptile(W1_d[:, ts(g, 768)]))
                        for m in range(6):
                            mt = 6 * g + m
                            ps = psF.tile([_P, 512], f32, tag="fc1")
                            for kt in range(_KC):
                                nc.tensor.matmul(
                                    ps[:], W1_sb[:, kt, ts(m, _P)],
                                    ln2T[:, kt, ts(c, 512)],
                                    start=(kt == 0), stop=(kt == _KC - 1))
                            nc.scalar.activation(
                                hT[:, mt, :], ps[:], AF.Gelu,
                                bias=b1_sb[:, mt:mt + 1], scale=1.0)
                    f2 = [psG.tile([_P, 512], f32, tag="fc2", name=f"fc2_{i}")
                          for i in range(_KC)]
                    for g in range(4):  # W2 row group: kt 6g..6g+5
                        W2_sb = dbl.tile([_P, _KC, _C], bf16, tag="w9")
                        nc.sync.dma_start(
                            W2_sb[:], ptile(W2_d[ts(g, 768), :]))
                        for k in range(6):
                            kt = 6 * g + k
                            for ot in range(_KC):
                                nc.tensor.matmul(
                                    f2[ot][:], W2_sb[:, k, ts(ot, _P)],
                                    hT[:, kt, :],
                                    start=(kt == 0), stop=(kt == _KM - 1))
                    for ot in range(_KC):
                        osb = sing.tile([_P, 512], f32, tag="outsb")
                        nc.vector.scalar_tensor_tensor(
                            osb[:], f2[ot][:], b2_sb[:, ot:ot + 1],
                            x1T[:, ot, ts(c, 512)], op0=OP.add, op1=OP.add)
                        nc.sync.dma_start(
                            outT_d[b, ts(ot, _P), ts(c, 512)], osb[:])

    _split_multi_waits(nc)
    return nc


_built_nc = None


def _prepare(inputs):
    global _built_nc
    import ml_dtypes
    from concourse import mybir

    bf = ml_dtypes.bfloat16
    mask_np = mybir.dt.np(
        mybir.dt.float8e4 if _MASK_DT == "fp8" else mybir.dt.bfloat16)

    x = np.asarray(inputs["x"], dtype=np.float32)
    ce = np.asarray(inputs["color_emb"], dtype=np.float32)
    mask = np.asarray(inputs["mask"])

    xT = np.ascontiguousarray(x.transpose(0, 2, 1)).astype(bf)       # [B,C,N]
    ceT = np.ascontiguousarray(ce.transpose(0, 2, 1)).astype(bf)     # [B,C,L]
    maskT = np.zeros((_B, _NLP, _N), dtype=mask_np)
    maskT[:, :_NL, :] = (mask.transpose(0, 2, 1) != 0).astype(mask_np)

    wz = {}
    for nm in ("Wq", "Wk", "Wv", "Wp", "W1", "W2"):
        wz[nm] = np.asarray(inputs[nm], dtype=np.float32).astype(bf)
    bz = {}
    for nm, key in (("bq", "bq"), ("bk", "bk"), ("bv", "bv"), ("bp", "bp"),
                    ("b1", "b1"), ("b2", "b2"), ("g1", "ln1_g"),
                    ("be1", "ln1_b"), ("g2", "ln2_g"), ("be2", "ln2_b")):
        bz[nm] = np.ascontiguousarray(
            np.asarray(inputs[key], dtype=np.float32))

    if _built_nc is None:
        _built_nc = _build()
    nc = _built_nc

    in_maps = []
    for i in range(_NCORES):
        s = slice(_BPC * i, _BPC * (i + 1))
        m = {"xT": np.ascontiguousarray(xT[s]),
             "ceT": np.ascontiguousarray(ceT[s]),
             "maskT": np.ascontiguousarray(maskT[s])}
        m.update(wz)
        m.update(bz)
        in_maps.append(m)
    return nc, in_maps


def kernel(**inputs):
    from concourse import bass_utils

    nc, in_maps = _prepare(inputs)
    res = bass_utils.run_bass_kernel_spmd(
        nc, in_maps, core_ids=list(range(_NCORES)), trace=_TRACE)
    globals()["last_results"] = res
    out = np.concatenate(
        [res.results[i]["outT"].transpose(0, 2, 1) for i in range(_NCORES)],
        axis=0)
    return np.ascontiguousarray(out.astype(np.float32))

